# revision 5
# baseline (speedup 1.0000x reference)
"""Masked attention (B=4, M=N=4096, D=64) on 8 Trainium2 NeuronCores.

Sharding: batch (4) x m-halves (2) -> 8 cores, no cross-core communication.
Each core computes out[m, :] = softmax(mask(q@k^T)/sqrt(d)) @ v for its
2048 q rows against the full 4096 k/v rows of its batch.

v2: the baseline (v1) was scalar-bound (64 exps x 1114ns = 71us busy,
packed back-to-back).  v2 spreads the elementwise work over three
engines by giving each 128-row n-chunk one of three flavors:

  A: ScalarE exp (PSUM->SBUF fp16) + DVE tensor_tensor mult by the fp16
     notmask (2x mode) -- the v1 path.
  B: PE applies the mask BEFORE exp by accumulating -240*maskT into the
     scores PSUM via an fp8 identity matmul (exp then maps masked scores
     to e^-30*e^s -> 0 in fp16).  No DVE work at all.
  F: DVE computes a Schraudolph fast-exp: t_i16 = round(a*S + b) written
     through an int16 bitcast of the fp16 p tile; the int16 bits ARE the
     fp16 exponent/mantissa of exp(s/8 - 3) (+-3% sawtooth, harmless
     under softmax normalization).  Mask via DVE mult.  No scalar work.

Other v2 changes: explicit tile_position on the row-packed QK pairs,
mask DMAs split across the sync and gpsimd queues, k/q DMAs issued first
on separate queues so the pipeline starts ~7us earlier.
"""

import numpy as np
import ml_dtypes
from contextlib import ExitStack

import concourse.bacc as bacc
import concourse.mybir as mybir
import concourse.tile as tile
from concourse.bass_utils import run_bass_kernel_spmd

B, M, N, D = 4, 4096, 4096, 64
NCORES = 8
M_LOC = M // 2        # q rows per core
MH = 1024             # m sub-block held in one PSUM accumulation
NCH = N // 128        # 32 n-chunks of 128
NPAIR = NCH // 2      # 16 chunk-pairs per m-half
SCALE = 1.0 / 8.0     # 1/sqrt(64)
EBIAS = -3.0
MASKC = 240.0         # fp8 mask subtractor: exp sees s/8 - 30 -> 0 in fp16
LOG2E = 1.4426950408889634
FE_A = SCALE * 1024.0 * LOG2E                  # fast-exp scale on raw s
FE_B = 1024.0 * (15.0 + EBIAS * LOG2E) - 44.0  # fp16-bits bias, centered
BF16 = mybir.dt.bfloat16
F32 = mybir.dt.float32
FP16 = mybir.dt.float16
FP8 = mybir.dt.float8e4
I16 = mybir.dt.int16

# pair flavor schedule per m-half (16 pairs): A=exp+mult, B=maskadd+exp,
# F=fastexp+mult.  Interleaved to keep every engine streaming.
PAIR_TYPES = ["A", "F", "B", "A", "A", "B", "F", "A", "B", "A", "F", "A",
              "B", "A", "F", "B"]
assert len(PAIR_TYPES) == NPAIR
N_A = PAIR_TYPES.count("A") * 2
N_B = PAIR_TYPES.count("B") * 2
N_F = PAIR_TYPES.count("F") * 2

_NC = None
LAST_RESULTS = None   # BassKernelResults of the most recent run (for profiling)
TRACE = False
TRACE_KW = {}
_RUN_IDX = 0


def _build_nc():
    nc = bacc.Bacc("TRN2", target_bir_lowering=False, debug=False,
                   num_devices=NCORES)
    qT = nc.dram_tensor("qT", [128, M_LOC], FP16, kind="ExternalInput").ap()
    kT = nc.dram_tensor("kT", [128, NPAIR * 128], FP16,
                        kind="ExternalInput").ap()
    vA = nc.dram_tensor("vA", [128, NCH * (D + 1)], FP16,
                        kind="ExternalInput").ap()
    nmT = nc.dram_tensor("nmT", [N, M_LOC], FP16, kind="ExternalInput").ap()
    m8T = nc.dram_tensor("m8T", [N, M_LOC], FP8, kind="ExternalInput").ap()
    id8 = nc.dram_tensor("id8", [128, 128], FP8, kind="ExternalInput").ap()
    # raw accumulator output: out^T with the softmax denominator in row 64;
    # the host does the (tiny) divide + transpose during unsharding
    o = nc.dram_tensor("oT", [2, D + 1, MH], F32, kind="ExternalOutput").ap()

    with tile.TileContext(nc) as tc, ExitStack() as ctx:
        const = ctx.enter_context(tc.tile_pool(name="const", bufs=1))
        m16pool = ctx.enter_context(tc.tile_pool(name="m16", bufs=5))
        m8pool = ctx.enter_context(tc.tile_pool(name="m8", bufs=3))
        epool = ctx.enter_context(tc.tile_pool(name="e", bufs=4))
        ppool = ctx.enter_context(tc.tile_pool(name="p", bufs=6))
        fpool = ctx.enter_context(tc.tile_pool(name="fin", bufs=2))
        spool = ctx.enter_context(tc.tile_pool(name="spsum", bufs=3, space="PSUM"))
        opool = ctx.enter_context(tc.tile_pool(name="opsum", bufs=1, space="PSUM"))

        # constants: k/q first (they gate the first QK), each on its own queue
        kT_s = const.tile([128, NPAIR * 128], FP16)
        nc.scalar.dma_start(kT_s[:], kT)
        qT_s = const.tile([128, M_LOC], FP16)
        nc.sync.dma_start(qT_s[:], qT)
        vA_s = const.tile([128, NCH * (D + 1)], FP16)
        nc.scalar.dma_start(vA_s[:], vA)
        id8_s = const.tile([128, 128], FP8)
        nc.scalar.dma_start(id8_s[:], id8)
        ebias = const.tile([128, 1], F32)
        nc.vector.memset(ebias[:], EBIAS)
        # warmup operand with no DMA dependency (starts right after preamble)
        wsrc = const.tile([128, 512], BF16)
        nc.vector.memset(wsrc[:], 1.0)

        # Dense back-to-back full-array (K=128) matmuls keep the PE HAM
        # clock-gate at 8/8 (results discarded into the first spool slot).
        def pe_keepalive(n, wu):
            for _ in range(n):
                nc.tensor.matmul(wu[:, 0:512], wsrc[:, 0:128], wsrc[:],
                                 start=True, stop=True)

        for h in range(2):
            if h == 0:
                wu = spool.tile([128, MH], F32, tag="s")
                pe_keepalive(10, wu)
            o_ps = opool.tile([D + 1, MH], F32)
            pv_pending = []

            def flush_pv():
                for ni, p in pv_pending:
                    vch = vA_s[:, ni * (D + 1):(ni + 1) * (D + 1)]
                    nc.tensor.matmul(o_ps[:, 0:512], vch, p[:, 0:512],
                                     start=(ni == 0), stop=(ni == NCH - 1))
                    nc.tensor.matmul(o_ps[:, 512:1024], vch, p[:, 512:1024],
                                     start=(ni == 0), stop=(ni == NCH - 1))
                pv_pending.clear()

            for pc in range(NPAIR):
                pt = PAIR_TYPES[pc]
                ni_e, ni_o = 2 * pc, 2 * pc + 1
                lhs_e = kT_s[0:64, pc * 128:(pc + 1) * 128]
                lhs_o = kT_s[64:128, pc * 128:(pc + 1) * 128]
                rhs_e = qT_s[0:64, h * MH:(h + 1) * MH]
                rhs_o = qT_s[64:128, h * MH:(h + 1) * MH]
                S_e = spool.tile([128, MH], F32, tag="s")
                S_o = spool.tile([128, MH], F32, tag="s")
                qk_stop = pt != "B"
                # interleave row-halves; explicit tile_position for packing
                nc.tensor.matmul(S_e[:, 0:512], lhs_e, rhs_e[:, 0:512],
                                 start=True, stop=qk_stop,
                                 tile_position=(0, 0))
                nc.tensor.matmul(S_o[:, 0:512], lhs_o, rhs_o[:, 0:512],
                                 start=True, stop=qk_stop,
                                 tile_position=(64, 0))
                nc.tensor.matmul(S_e[:, 512:1024], lhs_e, rhs_e[:, 512:1024],
                                 start=True, stop=qk_stop,
                                 tile_position=(0, 0))
                nc.tensor.matmul(S_o[:, 512:1024], lhs_o, rhs_o[:, 512:1024],
                                 start=True, stop=qk_stop,
                                 tile_position=(64, 0))
                if h == 1 and pc == 0:
                    pe_keepalive(8, wu)
                # PV of the PREVIOUS pair goes right after this pair's QK so
                # the PE never waits on the current pair's exp/mask chain.
                flush_pv()
                if pt == "B":
                    # accumulate -240*maskT into the scores (fp8 identity)
                    m8 = m8pool.tile([128, 2 * MH], FP8)
                    m8_src = m8T[ni_e * 128:(ni_e + 2) * 128,
                                 h * MH:(h + 1) * MH].rearrange(
                                     "(t p) m -> p t m", t=2)
                    nc.gpsimd.dma_start(
                        m8[:].rearrange("p (t m) -> p t m", t=2), m8_src)
                    for half, S in enumerate((S_e, S_o)):
                        mm = m8[:, half * MH:half * MH + MH]
                        nc.tensor.matmul(S[:, 0:512], id8_s[:],
                                         mm[:, 0:512], start=False, stop=True)
                        nc.tensor.matmul(S[:, 512:1024], id8_s[:],
                                         mm[:, 512:1024], start=False,
                                         stop=True)
                    for ni, S in ((ni_e, S_e), (ni_o, S_o)):
                        p = ppool.tile([128, MH], FP16)
                        nc.scalar.activation(p[:], S[:],
                                             mybir.ActivationFunctionType.Exp,
                                             bias=ebias[:], scale=SCALE)
                        pv_pending.append((ni, p))
                else:
                    # one DMA for the pair's notmask: [256, MH] -> [128, 2*MH]
                    nm = m16pool.tile([128, 2 * MH], FP16)
                    nm_src = nmT[ni_e * 128:(ni_e + 2) * 128,
                                 h * MH:(h + 1) * MH].rearrange(
                                     "(t p) m -> p t m", t=2)
                    dmaq = nc.sync if (pc % 2 == 0) else nc.gpsimd
                    dmaq.dma_start(nm[:].rearrange("p (t m) -> p t m", t=2),
                                   nm_src)
                    for half, (ni, S) in enumerate(((ni_e, S_e), (ni_o, S_o))):
                        e = epool.tile([128, MH], FP16)
                        if pt == "A":
                            nc.scalar.activation(
                                e[:], S[:], mybir.ActivationFunctionType.Exp,
                                bias=ebias[:], scale=SCALE)
                        else:  # F: DVE fast-exp via int16 bitcast
                            nc.vector.tensor_scalar(
                                e[:].bitcast(I16), S[:], FE_A, FE_B,
                                mybir.AluOpType.mult, mybir.AluOpType.add)
                        p = ppool.tile([128, MH], FP16)
                        nc.vector.tensor_mul(p[:], e[:],
                                             nm[:, half * MH:(half + 1) * MH])
                        pv_pending.append((ni, p))
            flush_pv()
            oT = fpool.tile([D + 1, MH], F32)
            nc.vector.tensor_copy(oT[:, 0:MH // 2], o_ps[:, 0:MH // 2])
            nc.sync.dma_start(o[h, :, 0:MH // 2], oT[:, 0:MH // 2])
            nc.vector.tensor_copy(oT[:, MH // 2:MH], o_ps[:, MH // 2:MH])
            nc.sync.dma_start(o[h, :, MH // 2:MH], oT[:, MH // 2:MH])
    nc.compile()
    return nc


def _get_nc():
    global _NC
    if _NC is None:
        _NC = _build_nc()
    return _NC


_ID8 = None


def _prep_core(q, k, v, mask, b, j):
    global _ID8
    qs = q[b, j * M_LOC:(j + 1) * M_LOC, :]
    qT = np.ascontiguousarray(qs.T).astype(np.float16)    # [64, 2048]
    qTp = np.concatenate([qT, qT], axis=0)                # [128, 2048]
    kTf = np.ascontiguousarray(k[b].T).astype(np.float16) # [64, 4096]
    kTp = np.empty((128, NPAIR * 128), np.float16)
    kTr = kTf.reshape(64, NCH, 128)
    kTp[0:64] = kTr[:, 0::2, :].reshape(64, -1)
    kTp[64:128] = kTr[:, 1::2, :].reshape(64, -1)
    vb = v[b]                                             # [4096, 64]
    vA = np.empty((128, NCH * (D + 1)), np.float16)
    vAr = vA.reshape(128, NCH, D + 1)
    vAr[:, :, :D] = vb.reshape(NCH, 128, D).transpose(1, 0, 2).astype(np.float16)
    vAr[:, :, D] = np.float16(1.0)
    mT = np.ascontiguousarray(mask[b, j * M_LOC:(j + 1) * M_LOC, :].T)
    nmT = (~mT).astype(np.float16)
    m8T = mT.astype(ml_dtypes.float8_e4m3)
    if _ID8 is None:
        _ID8 = (np.eye(128, dtype=np.float32) * -MASKC).astype(
            ml_dtypes.float8_e4m3)
    return {"qT": qTp, "kT": kTp, "vA": vA, "nmT": nmT, "m8T": m8T,
            "id8": _ID8}


def kernel(q, k, v, mask):
    global LAST_RESULTS
    q = np.asarray(q, dtype=np.float32)
    k = np.asarray(k, dtype=np.float32)
    v = np.asarray(v, dtype=np.float32)
    mask = np.asarray(mask)
    nc = _get_nc()
    in_maps = [_prep_core(q, k, v, mask, c // 2, c % 2) for c in range(NCORES)]
    kw = dict(TRACE_KW)
    if "tmpdir" in kw:
        import os
        global _RUN_IDX
        _RUN_IDX += 1
        kw["tmpdir"] = os.path.join(kw["tmpdir"], f"run{_RUN_IDX}")
        os.makedirs(kw["tmpdir"], exist_ok=True)
    res = run_bass_kernel_spmd(nc, in_maps, core_ids=list(range(NCORES)),
                               trace=TRACE, **kw)
    LAST_RESULTS = res
    out = np.empty((B, M, D), np.float32)
    for c in range(NCORES):
        b, j = divmod(c, 2)
        oT = res.results[c]["oT"]                      # [2, 65, MH]
        for h in range(2):
            blk = oT[h, :D, :] / oT[h, D, :]           # [64, MH]
            lo = j * M_LOC + h * MH
            out[b, lo:lo + MH, :] = blk.T
    return out


# revision 6
# speedup vs baseline: 1.0731x; 1.0731x over previous
"""Masked attention (B=4, M=N=4096, D=64) on 8 Trainium2 NeuronCores.

Sharding: batch (4) x m-halves (2) -> 8 cores, no cross-core communication.
Each core computes out[m, :] = softmax(mask(q@k^T)/sqrt(d)) @ v for its
2048 q rows against the full 4096 k/v rows of its batch.

v3 architecture (per core): scores are computed TRANSPOSED in PSUM
pair-tiles S [128n, 1024] covering an (even, odd) n-chunk pair at one
512-wide m-block.  The elementwise exp+mask work is spread over three
engines by pair flavor:

  A: ScalarE exp (one [128,1024] ACTIVATE per pair) + DVE mult by the
     fp16 notmask (2x mode).
  B: PE accumulates -240*maskT into the scores via an fp8 identity
     matmul before exp (masked scores -> e^-30*e^s -> 0 in fp16); the
     exp is the only elementwise op -- no DVE work.
  F: DVE Schraudolph fast-exp: t_i16 = round(a*S + b) written through an
     int16 bitcast of the fp16 tile; the int16 bits are the fp16
     encoding of exp(s/8-3) (+-3% sawtooth, which softmax normalization
     averages away; the worst-core rel err of the mix is simulated
     offline and kept under the harness gate).  Mask via DVE mult.

MH=512 m-blocks keep PSUM at 3 pair-tiles of runway + a double-buffered
output accumulator (8 banks exactly), so the PE never stalls on the
elementwise stage.  PV uses the ones-augmented v so row 64 of the
accumulator is the softmax denominator; the host divides + transposes
while unsharding.
"""

import numpy as np
import ml_dtypes
from contextlib import ExitStack

import concourse.bacc as bacc
import concourse.mybir as mybir
import concourse.tile as tile
from concourse.bass_utils import run_bass_kernel_spmd

B, M, N, D = 4, 4096, 4096, 64
NCORES = 8
M_LOC = M // 2        # q rows per core
MH = 512              # m sub-block held in one PSUM accumulation
NMB = M_LOC // MH     # 4 m-blocks
NCH = N // 128        # 32 n-chunks of 128
NPAIR = NCH // 2      # 16 chunk-pairs
SCALE = 1.0 / 8.0     # 1/sqrt(64)
EBIAS = -3.0
MASKC = 240.0         # fp8 mask subtractor: exp sees s/8 - 30 -> 0 in fp16
LOG2E = 1.4426950408889634
FE_A = SCALE * 1024.0 * LOG2E                  # fast-exp scale on raw s
FE_B = 1024.0 * (15.0 + EBIAS * LOG2E) - 44.0  # fp16-bits bias, centered
BF16 = mybir.dt.bfloat16
F32 = mybir.dt.float32
FP16 = mybir.dt.float16
FP8 = mybir.dt.float8e4
I16 = mybir.dt.int16

# pair flavor schedule (16 n-pairs, same at every m-block):
# A=exp+mult, B=maskadd+exp, F=fastexp+mult.
PAIR_TYPES = ["A", "F", "B", "A", "A", "B", "F", "A",
              "B", "A", "F", "A", "A", "F", "A", "B"]
assert len(PAIR_TYPES) == NPAIR

_NC = None
LAST_RESULTS = None   # BassKernelResults of the most recent run (for profiling)
TRACE = False
TRACE_KW = {}
_RUN_IDX = 0


def _build_nc():
    nc = bacc.Bacc("TRN2", target_bir_lowering=False, debug=False,
                   num_devices=NCORES)
    qT = nc.dram_tensor("qT", [128, M_LOC], FP16, kind="ExternalInput").ap()
    kT = nc.dram_tensor("kT", [128, NPAIR * 128], FP16,
                        kind="ExternalInput").ap()
    vA = nc.dram_tensor("vA", [128, NCH * (D + 1)], FP16,
                        kind="ExternalInput").ap()
    nmT = nc.dram_tensor("nmT", [N, M_LOC], FP16, kind="ExternalInput").ap()
    m8T = nc.dram_tensor("m8T", [N, M_LOC], FP8, kind="ExternalInput").ap()
    id8 = nc.dram_tensor("id8", [128, 128], FP8, kind="ExternalInput").ap()
    # raw accumulator output: out^T with the softmax denominator in row 64;
    # the host does the (tiny) divide + transpose during unsharding
    o = nc.dram_tensor("oT", [NMB, D + 1, MH], F32, kind="ExternalOutput").ap()

    with tile.TileContext(nc) as tc, ExitStack() as ctx:
        const = ctx.enter_context(tc.tile_pool(name="const", bufs=1))
        m16pool = ctx.enter_context(tc.tile_pool(name="m16", bufs=6))
        m8pool = ctx.enter_context(tc.tile_pool(name="m8", bufs=4))
        epool = ctx.enter_context(tc.tile_pool(name="e", bufs=4))
        ppool = ctx.enter_context(tc.tile_pool(name="p", bufs=8))
        fpool = ctx.enter_context(tc.tile_pool(name="fin", bufs=2))
        spool = ctx.enter_context(tc.tile_pool(name="spsum", bufs=3, space="PSUM"))
        opool = ctx.enter_context(tc.tile_pool(name="opsum", bufs=2, space="PSUM"))

        # constants: k/q first (they gate the first QK), each on its own queue
        kT_s = const.tile([128, NPAIR * 128], FP16)
        nc.scalar.dma_start(kT_s[:], kT)
        qT_s = const.tile([128, M_LOC], FP16)
        nc.sync.dma_start(qT_s[:], qT)
        id8_s = const.tile([128, 128], FP8)
        nc.sync.dma_start(id8_s[:], id8)
        vA_s = const.tile([128, NCH * (D + 1)], FP16)
        nc.scalar.dma_start(vA_s[:], vA)
        ebias = const.tile([128, 1], F32)
        nc.vector.memset(ebias[:], EBIAS)
        # warmup operand with no DMA dependency (starts right after preamble)
        wsrc = const.tile([128, 512], BF16)
        nc.vector.memset(wsrc[:], 1.0)

        # Dense back-to-back full-array (K=128) matmuls keep the PE HAM
        # clock-gate at 8/8 (results discarded into the first spool slot).
        wu = spool.tile([128, 2 * MH], F32, tag="s")
        for _ in range(10):
            nc.tensor.matmul(wu[:, 0:512], wsrc[:, 0:128], wsrc[:],
                             start=True, stop=True)

        for mb in range(NMB):
            o_ps = opool.tile([D + 1, MH], F32)
            pv_pending = []

            def flush_pv():
                for ni, pr in pv_pending:
                    vch = vA_s[:, ni * (D + 1):(ni + 1) * (D + 1)]
                    nc.tensor.matmul(o_ps[:], vch, pr,
                                     start=(ni == 0), stop=(ni == NCH - 1))
                pv_pending.clear()

            for pc in range(NPAIR):
                pt = PAIR_TYPES[pc]
                ni_e, ni_o = 2 * pc, 2 * pc + 1
                lhs_e = kT_s[0:64, pc * 128:(pc + 1) * 128]
                lhs_o = kT_s[64:128, pc * 128:(pc + 1) * 128]
                rhs_e = qT_s[0:64, mb * MH:(mb + 1) * MH]
                rhs_o = qT_s[64:128, mb * MH:(mb + 1) * MH]
                S = spool.tile([128, 2 * MH], F32, tag="s")
                qk_stop = pt != "B"
                # adjacent emission of the row-packed pair for dual-issue
                nc.tensor.matmul(S[:, 0:MH], lhs_e, rhs_e,
                                 start=True, stop=qk_stop,
                                 tile_position=(0, 0))
                nc.tensor.matmul(S[:, MH:2 * MH], lhs_o, rhs_o,
                                 start=True, stop=qk_stop,
                                 tile_position=(64, 0))
                # PV of the PREVIOUS pair goes right after this pair's QK so
                # the PE never waits on the current pair's exp/mask chain.
                flush_pv()
                if pt == "B":
                    # accumulate -240*maskT into the scores (fp8 identity)
                    m8 = m8pool.tile([128, 2 * MH], FP8)
                    m8_src = m8T[ni_e * 128:(ni_e + 2) * 128,
                                 mb * MH:(mb + 1) * MH].rearrange(
                                     "(t p) m -> p t m", t=2)
                    nc.gpsimd.dma_start(
                        m8[:].rearrange("p (t m) -> p t m", t=2), m8_src)
                    nc.tensor.matmul(S[:, 0:MH], id8_s[:], m8[:, 0:MH],
                                     start=False, stop=True)
                    nc.tensor.matmul(S[:, MH:2 * MH], id8_s[:], m8[:, MH:2 * MH],
                                     start=False, stop=True)
                    p = ppool.tile([128, 2 * MH], FP16)
                    nc.scalar.activation(p[:], S[:],
                                         mybir.ActivationFunctionType.Exp,
                                         bias=ebias[:], scale=SCALE)
                else:
                    nm = m16pool.tile([128, 2 * MH], FP16)
                    nm_src = nmT[ni_e * 128:(ni_e + 2) * 128,
                                 mb * MH:(mb + 1) * MH].rearrange(
                                     "(t p) m -> p t m", t=2)
                    dmaq = nc.sync if (pc % 2 == 0) else nc.gpsimd
                    dmaq.dma_start(nm[:].rearrange("p (t m) -> p t m", t=2),
                                   nm_src)
                    e = epool.tile([128, 2 * MH], FP16)
                    if pt == "A":
                        nc.scalar.activation(
                            e[:], S[:], mybir.ActivationFunctionType.Exp,
                            bias=ebias[:], scale=SCALE)
                    else:  # F: DVE fast-exp via int16 bitcast
                        nc.vector.tensor_scalar(
                            e[:].bitcast(I16), S[:], FE_A, FE_B,
                            mybir.AluOpType.mult, mybir.AluOpType.add)
                    p = ppool.tile([128, 2 * MH], FP16)
                    nc.vector.tensor_mul(p[:], e[:], nm[:])
                pv_pending.append((ni_e, p[:, 0:MH]))
                pv_pending.append((ni_o, p[:, MH:2 * MH]))
            flush_pv()
            oT = fpool.tile([D + 1, MH], F32)
            nc.vector.tensor_copy(oT[:], o_ps[:])
            nc.sync.dma_start(o[mb], oT[:])
    nc.compile()
    return nc


def _get_nc():
    global _NC
    if _NC is None:
        _NC = _build_nc()
    return _NC


_ID8 = None


def _prep_core(q, k, v, mask, b, j):
    global _ID8
    qs = q[b, j * M_LOC:(j + 1) * M_LOC, :]
    qT = np.ascontiguousarray(qs.T).astype(np.float16)    # [64, 2048]
    qTp = np.concatenate([qT, qT], axis=0)                # [128, 2048]
    kTf = np.ascontiguousarray(k[b].T).astype(np.float16) # [64, 4096]
    kTp = np.empty((128, NPAIR * 128), np.float16)
    kTr = kTf.reshape(64, NCH, 128)
    kTp[0:64] = kTr[:, 0::2, :].reshape(64, -1)
    kTp[64:128] = kTr[:, 1::2, :].reshape(64, -1)
    vb = v[b]                                             # [4096, 64]
    vA = np.empty((128, NCH * (D + 1)), np.float16)
    vAr = vA.reshape(128, NCH, D + 1)
    vAr[:, :, :D] = vb.reshape(NCH, 128, D).transpose(1, 0, 2).astype(np.float16)
    vAr[:, :, D] = np.float16(1.0)
    mT = np.ascontiguousarray(mask[b, j * M_LOC:(j + 1) * M_LOC, :].T)
    nmT = (~mT).astype(np.float16)
    m8T = mT.astype(ml_dtypes.float8_e4m3)
    if _ID8 is None:
        _ID8 = (np.eye(128, dtype=np.float32) * -MASKC).astype(
            ml_dtypes.float8_e4m3)
    return {"qT": qTp, "kT": kTp, "vA": vA, "nmT": nmT, "m8T": m8T,
            "id8": _ID8}


def kernel(q, k, v, mask):
    global LAST_RESULTS, _RUN_IDX
    q = np.asarray(q, dtype=np.float32)
    k = np.asarray(k, dtype=np.float32)
    v = np.asarray(v, dtype=np.float32)
    mask = np.asarray(mask)
    nc = _get_nc()
    in_maps = [_prep_core(q, k, v, mask, c // 2, c % 2) for c in range(NCORES)]
    kw = dict(TRACE_KW)
    if "tmpdir" in kw:
        import os
        _RUN_IDX += 1
        kw["tmpdir"] = os.path.join(kw["tmpdir"], f"run{_RUN_IDX}")
        os.makedirs(kw["tmpdir"], exist_ok=True)
    res = run_bass_kernel_spmd(nc, in_maps, core_ids=list(range(NCORES)),
                               trace=TRACE, **kw)
    LAST_RESULTS = res
    out = np.empty((B, M, D), np.float32)
    for c in range(NCORES):
        b, j = divmod(c, 2)
        oT = res.results[c]["oT"]                      # [NMB, 65, MH]
        for mb in range(NMB):
            blk = oT[mb, :D, :] / oT[mb, D, :]         # [64, MH]
            lo = j * M_LOC + mb * MH
            out[b, lo:lo + MH, :] = blk.T
    return out
